# revision 30
# baseline (speedup 1.0000x reference)
"""Trainium2 Bass kernel for nn_BaseAttention (sliding-window attention).

Full-input contract: kernel(x, Wqkv) -> [B, T, C] float32.

Sharding (8 cores): data-parallel over B (2) x tensor-parallel over head
groups (16 heads -> 4 groups of 4). Core c handles batch c//4, head group
c%4. Each core computes its QKV projection slice (768 of 3072 output rows)
and banded attention for its 4 heads; outputs are disjoint channel slices
of the final [B, T, C] tensor, so no collectives are needed.

All matmul operands are bf16 (fp32 PSUM accumulation); the correctness
gate (2e-2 max rel err) has ~6x margin at this precision.

Device-side structure (per core):
  Projection: q,k in [d, t] orientation (w-chunk stationary, N=512 moving),
  v in [t, d] orientation (x-chunk stationary, N=256) packed per key block
  as [v | 1] so P^T @ [v | 1] yields output + softmax denominator at once.

  Attention, key-chunk stationary: for key chunk jb, ONE matmul per head
  computes scores^T [128 keys, 384 queries] against the 3 query blocks
  that can see chunk jb (K=64 head pairs run concurrently via row tiling
  at base partitions 0/64). All 4 heads write one 4-bank PSUM tile, so a
  single ACT exp produces P^T for all heads; the sliding-window mask is a
  0/1 multiply on the two 128-col edge slots only. P@V is query-major:
  out[q, 0:65] = sum_jb P^T[jb].T @ [v|1][jb] -- no PE transposes anywhere.
  Normalization: batched reciprocal of the 4 denominator columns + one
  broadcast tensor-tensor multiply during PSUM eviction.

  Emission interleaves attention chunk-steps between projection m-groups
  of the NEXT t-slice so the PE queue never drains (HAM stays at 8/8) and
  the ACT FIFO (projection evictions + exps) never back-pressures the
  projection PSUM ping-pong.
"""

import os
import sys

import numpy as np

if "/opt/trn_rl_repo" not in sys.path:
    sys.path.insert(0, "/opt/trn_rl_repo")

B, T, C = 2, 2048, 1024
HEADS = 16
D = C // HEADS  # 64
WINDOW = 128
N_CORES = 8
HPC = HEADS // 4  # heads per core (4)
OPC = 3 * HPC * D  # projection output rows per core (768)

PDT_NAME = os.environ.get("SA_PDT", "bf16")

_PROGRAM_CACHE = {}


def _build_program(pdt_name):
    import concourse.mybir as mybir
    from concourse import bacc
    import concourse.tile as tile
    from contextlib import ExitStack

    f32 = mybir.dt.float32
    bf16 = mybir.dt.bfloat16
    Exp = mybir.ActivationFunctionType.Exp

    nc = bacc.Bacc()
    xT_d = nc.declare_dram_parameter("xT", [C, T], bf16, isOutput=False)
    wT_d = nc.declare_dram_parameter("wT", [C, OPC], bf16, isOutput=False)
    msk_d = nc.declare_dram_parameter("msk", [128, 2, 128], bf16, isOutput=False)
    out_d = nc.declare_dram_parameter("out", [T, HPC * D], f32, isOutput=True)

    CC = C // 128  # 8 contraction chunks
    TS = 512  # max projection t-slice (PSUM bank cap)
    NB = T // 128  # 16 query/key blocks
    # variable t-slices: small first (compute starts sooner after less DMA)
    # and small last (fewer attention chunks left for the serial tail)
    SLICES = [(0, 256), (256, 256), (512, 512), (1024, 512), (1536, 256), (1792, 256)]

    with ExitStack() as ctx:
        tc = ctx.enter_context(tile.TileContext(nc))
        const = ctx.enter_context(tc.tile_pool(name="const", bufs=1))
        xpool = ctx.enter_context(tc.tile_pool(name="xp", bufs=3))
        ppool = ctx.enter_context(tc.tile_pool(name="pp", bufs=5))
        opool = ctx.enter_context(tc.tile_pool(name="op", bufs=2))
        rpool = ctx.enter_context(tc.tile_pool(name="rp", bufs=2))
        pj_ps = ctx.enter_context(tc.tile_pool(name="pjps", bufs=2, space="PSUM"))
        sc_ps = ctx.enter_context(tc.tile_pool(name="scps", bufs=2, space="PSUM"))
        ov_ps = ctx.enter_context(tc.tile_pool(name="ovps", bufs=2, space="PSUM"))

        # ---- constants / persistent SBUF ----
        w_sb = const.tile([128, CC, OPC], bf16)
        wT_r = wT_d.rearrange("(cc p) o -> p cc o", p=128)
        msk_sb = const.tile([128, 2, 128], bf16)
        # HAM prewarm: zeros tile + dummy matmuls keep the PE busy during
        # input staging so the clock gate is at 8/8 when real work arrives
        z_sb = const.tile([128, TS], bf16)
        nc.vector.memset(z_sb, 0.0)

        q_sb = const.tile([128, 2, T], bf16)
        k_sb = const.tile([128, 2, T], bf16)
        # v packed per (key block, head) with a trailing ones column
        v_sb = const.tile([128, NB, HPC, D + 1], bf16)
        nc.vector.memset(v_sb[:, :, :, D:D + 1], 1.0)

        xT_r = xT_d.rearrange("(cc p) t -> p cc t", p=128)

        xs_tiles = {}

        def emit_x_dma(s):
            st, ts = SLICES[s]
            xs = xpool.tile([128, CC, TS], bf16, tag="xs")
            if s == 0:
                nc.sync.dma_start(out=xs[:, 0:4, 0:ts], in_=xT_r[:, 0:4, st:st + ts])
                nc.sync.dma_start(out=xs[:, 4:8, 0:ts], in_=xT_r[:, 4:8, st:st + ts])
            else:
                nc.sync.dma_start(out=xs[:, :, 0:ts], in_=xT_r[:, :, st:st + ts])
            xs_tiles[s] = xs

        def emit_proj_qk_group(s, m):
            """One output m-tile (128 rows of q|k) for t-slice s."""
            st, ts = SLICES[s]
            xs = xs_tiles[s]
            ps = pj_ps.tile([128, TS], f32, tag="pj")
            for c in range(CC):
                nc.tensor.matmul(
                    ps[:, 0:ts],
                    lhsT=w_sb[:, c, m * 128:(m + 1) * 128],
                    rhs=xs[:, c, 0:ts],
                    start=(c == 0),
                    stop=(c == CC - 1),
                )
            dst = (q_sb if m < 2 else k_sb)[:, m % 2, st:st + ts]
            # balance PSUM evictions: q tiles on ACT, k tiles on DVE
            if m < 2:
                nc.scalar.copy(dst, ps[:, 0:ts])
            else:
                nc.vector.tensor_copy(dst, ps[:, 0:ts])

        def emit_proj_v_group(s, t4):
            """One key block (128 tokens) of v for t-slice s."""
            st, ts = SLICES[s]
            xs = xs_tiles[s]
            ps = pj_ps.tile([128, TS], f32, tag="pj")
            pv = ps[:, 0:HPC * D]
            for c in range(CC):
                nc.tensor.matmul(
                    pv,
                    lhsT=xs[:, c, t4 * 128:(t4 + 1) * 128],
                    rhs=w_sb[:, c, 2 * HPC * D:3 * HPC * D],
                    start=(c == 0),
                    stop=(c == CC - 1),
                )
            tb = st // 128 + t4
            nc.vector.tensor_copy(
                v_sb[:, tb, :, 0:D], pv.rearrange("p (h d) -> p h d", h=HPC)
            )

        p_tiles = {}

        def emit_attn_chunk(jb):
            """scores^T + exp + mask for key chunk jb (all 4 heads)."""
            blo = max(jb - 1, 0)
            bhi = min(jb + 2, NB)
            qlo, qhi = blo * 128, bhi * 128
            n = qhi - qlo
            off = 128 if jb == 0 else 0  # slot offset * 128
            slo = 1 if jb == 0 else 0
            shi = slo + (n // 128)
            p_t = ppool.tile([128, HPC, 3, 128], bf16, tag="p")
            # separate PSUM tile + exp per head pair (mt): halves the
            # score->exp->next-score serialization latency in the tail
            for mt in range(2):
                sct = sc_ps.tile([128, 2, TS], f32, tag="sc")
                for ph in range(2):
                    nc.tensor.matmul(
                        sct[:, ph, off:off + n],
                        lhsT=k_sb[ph * 64:(ph + 1) * 64, mt, jb * 128:(jb + 1) * 128],
                        rhs=q_sb[ph * 64:(ph + 1) * 64, mt, qlo:qhi],
                        start=True,
                        stop=True,
                    )
                nc.scalar.activation(
                    p_t[:, 2 * mt:2 * mt + 2, slo:shi, :],
                    sct[:, :, off:off + n].rearrange("p h (s q) -> p h s q", q=128),
                    Exp,
                )
            # window mask on the edge slots (slot 0: queries of block jb-1,
            # slot 2: queries of block jb+1); middle slot always in-window
            if jb == 0:
                nc.vector.tensor_mul(
                    p_t[:, :, 2, :],
                    p_t[:, :, 2, :],
                    msk_sb[:, 1:2, :].broadcast_to([128, HPC, 128]),
                )
            elif jb == NB - 1:
                nc.vector.tensor_mul(
                    p_t[:, :, 0, :],
                    p_t[:, :, 0, :],
                    msk_sb[:, 0:1, :].broadcast_to([128, HPC, 128]),
                )
            else:
                nc.vector.tensor_mul(
                    p_t[:, :, 0:3:2, :],
                    p_t[:, :, 0:3:2, :],
                    msk_sb[:, :, :].unsqueeze(1).broadcast_to([128, HPC, 2, 128]),
                )
            p_tiles[jb] = p_t

        def emit_pv_block(i):
            """P @ [v|1] for query block i + normalize + store."""
            jbs = [jb for jb in (i - 1, i, i + 1) if 0 <= jb < NB]
            ov = ov_ps.tile([128, HPC, D + 1], f32, tag="ov")
            for h in range(HPC):
                for ci, jb in enumerate(jbs):
                    slot = i - jb + 1
                    nc.tensor.matmul(
                        ov[:, h, :],
                        lhsT=p_tiles[jb][:, h, slot, :],
                        rhs=v_sb[:, jb, h, :],
                        start=(ci == 0),
                        stop=(ci == len(jbs) - 1),
                    )
            r_t = rpool.tile([128, HPC], f32, tag="r")
            nc.vector.reciprocal(r_t, ov[:, :, D])
            o_t = opool.tile([128, HPC, D], f32, tag="o")
            if i == NB - 2:
                # second-to-last block normalizes on ACT so the final two
                # blocks' evictions overlap instead of serializing on DVE
                for h in range(HPC):
                    nc.scalar.mul(o_t[:, h, :], ov[:, h, 0:D], r_t[:, h:h + 1])
            else:
                nc.vector.tensor_mul(
                    o_t,
                    ov[:, :, 0:D],
                    r_t[:, :].unsqueeze(2).broadcast_to([128, HPC, D]),
                )
            eng = nc.gpsimd if i % 2 == 0 else nc.sync
            eng.dma_start(out=out_d[i * 128:(i + 1) * 128, :], in_=o_t)

        # ---- schedule ----
        # attention chunk-steps for key chunk jb are interleaved into the
        # projection of the first slice after jb's inputs are complete;
        # the last 3 chunks run after all projection work.
        attn_of_slice = {
            1: [0], 2: [1, 2], 3: [3, 4, 5, 6], 4: [7, 8, 9, 10], 5: [11, 12],
        }
        tail_chunks = [13, 14, 15]
        PV_LAG = 3

        next_pv = [0]

        def emit_attn_step(jb):
            emit_attn_chunk(jb)
            i = jb - PV_LAG
            while next_pv[0] <= i:
                emit_pv_block(next_pv[0])
                next_pv[0] += 1

        # trickle-fed staging: w goes through the Scalar DGE in
        # consumption-ordered pieces while Sync streams x, so the first
        # matmul only waits for ~0.4MB; the cold-clock period then
        # self-paces consumption to the DMA ramp
        nc.scalar.dma_start(out=w_sb[:, 0:4, 0:128], in_=wT_r[:, 0:4, 0:128])
        emit_x_dma(0)
        nc.scalar.dma_start(out=w_sb[:, 4:8, 0:128], in_=wT_r[:, 4:8, 0:128])
        emit_x_dma(1)
        nc.scalar.dma_start(out=w_sb[:, :, 128:384], in_=wT_r[:, :, 128:384])
        nc.scalar.dma_start(out=w_sb[:, :, 384:768], in_=wT_r[:, :, 384:768])
        nc.gpsimd.dma_start(out=msk_sb, in_=msk_d[:, :, :])
        # short HAM prewarm; the real stream takes over while still cold
        warm = sc_ps.tile([128, 2, TS], f32, tag="sc")
        for _ in range(4):
            nc.tensor.matmul(
                warm[:, 0, :], lhsT=z_sb[:, 0:128], rhs=z_sb, start=True, stop=True
            )

        for s in range(len(SLICES)):
            if s + 2 < len(SLICES):
                emit_x_dma(s + 2)
            steps = list(attn_of_slice.get(s, []))
            nsteps = len(steps)
            nt4 = SLICES[s][1] // 128
            groups = [("qk", m) for m in range(4)] + [("v", t4) for t4 in range(nt4)]
            ng = len(groups)
            for gi, (kind, idx) in enumerate(groups):
                if kind == "qk":
                    emit_proj_qk_group(s, idx)
                else:
                    emit_proj_v_group(s, idx)
                # spread this slice's attention steps evenly between groups
                target = (nsteps * (gi + 1) + ng - 1) // ng
                while len(steps) > nsteps - target:
                    emit_attn_step(steps.pop(0))
            while steps:
                emit_attn_step(steps.pop(0))
        for jb in tail_chunks:
            emit_attn_step(jb)
        while next_pv[0] < NB:
            emit_pv_block(next_pv[0])
            next_pv[0] += 1

    nc.compile()
    return nc


def _host_inputs(x, Wqkv):
    """Per-core input maps: shard batch x head-group, pre-transpose, bf16."""
    import concourse.mybir as mybir

    bf16_np = mybir.dt.np(mybir.dt.bfloat16)
    scale = float(D) ** -0.5
    r = np.arange(128, dtype=np.float32)[:, None]
    ci = np.arange(128, dtype=np.float32)[None, :]
    # slot 0 (queries of block jb-1): allowed iff key row r <= query col c
    # slot 2 (queries of block jb+1): allowed iff c <= r
    msk = np.stack(
        [
            (ci >= r).astype(np.float32),
            (ci <= r).astype(np.float32),
        ],
        axis=1,
    ).astype(bf16_np)  # [128, 2, 128]

    x = np.asarray(x, dtype=np.float32)
    Wqkv = np.asarray(Wqkv, dtype=np.float32)
    xT = [np.ascontiguousarray(x[b].T).astype(bf16_np) for b in range(B)]
    in_maps = []
    for core in range(N_CORES):
        b, hg = divmod(core, N_CORES // B)
        rows = slice(hg * HPC * D, (hg + 1) * HPC * D)
        wcat = np.concatenate(
            [
                Wqkv[0 * C:1 * C][rows] * scale,
                Wqkv[1 * C:2 * C][rows],
                Wqkv[2 * C:3 * C][rows],
            ],
            axis=0,
        )
        in_maps.append(
            {
                "xT": xT[b],
                "wT": np.ascontiguousarray(wcat.T).astype(bf16_np),
                "msk": msk,
            }
        )
    return in_maps


def _gather(results):
    out = np.empty((B, T, C), dtype=np.float32)
    for core in range(N_CORES):
        b, hg = divmod(core, N_CORES // B)
        out[b, :, hg * HPC * D:(hg + 1) * HPC * D] = results[core]["out"]
    return out


def kernel(x, Wqkv):
    from concourse.bass_utils import run_bass_kernel_spmd

    key = PDT_NAME
    if key not in _PROGRAM_CACHE:
        _PROGRAM_CACHE[key] = _build_program(key)
    nc = _PROGRAM_CACHE[key]
    in_maps = _host_inputs(x, Wqkv)
    res = run_bass_kernel_spmd(nc, in_maps, list(range(N_CORES)))
    return _gather(res.results)


# revision 32
# speedup vs baseline: 1.2019x; 1.2019x over previous
"""Trainium2 Bass kernel for nn_BaseAttention (sliding-window attention).

Full-input contract: kernel(x, Wqkv) -> [B, T, C] float32.

Sharding (8 cores): data-parallel over B (2) x tensor-parallel over head
groups (16 heads -> 4 groups of 4). Core c handles batch c//4, head group
c%4. Each core computes its QKV projection slice (768 of 3072 output rows)
and banded attention for its 4 heads; outputs are disjoint channel slices
of the final [B, T, C] tensor, so no collectives are needed.

All matmul operands are bf16 (fp32 PSUM accumulation); the correctness
gate (2e-2 max rel err) has ~6x margin at this precision.

Device-side structure (per core):
  Projection: q,k in [d, t] orientation (w-chunk stationary, N=512 moving),
  v in [t, d] orientation (x-chunk stationary, N=256) packed per key block
  as [v | 1] so P^T @ [v | 1] yields output + softmax denominator at once.

  Attention, key-chunk stationary: for key chunk jb, ONE matmul per head
  computes scores^T [128 keys, 384 queries] against the 3 query blocks
  that can see chunk jb (K=64 head pairs run concurrently via row tiling
  at base partitions 0/64). All 4 heads write one 4-bank PSUM tile, so a
  single ACT exp produces P^T for all heads; the sliding-window mask is a
  0/1 multiply on the two 128-col edge slots only. P@V is query-major:
  out[q, 0:65] = sum_jb P^T[jb].T @ [v|1][jb] -- no PE transposes anywhere.
  Normalization: batched reciprocal of the 4 denominator columns + one
  broadcast tensor-tensor multiply during PSUM eviction.

  Emission interleaves attention chunk-steps between projection m-groups
  of the NEXT t-slice so the PE queue never drains (HAM stays at 8/8) and
  the ACT FIFO (projection evictions + exps) never back-pressures the
  projection PSUM ping-pong.
"""

import os
import sys

import numpy as np

if "/opt/trn_rl_repo" not in sys.path:
    sys.path.insert(0, "/opt/trn_rl_repo")

B, T, C = 2, 2048, 1024
HEADS = 16
D = C // HEADS  # 64
WINDOW = 128
N_CORES = 8
HPC = HEADS // 4  # heads per core (4)
OPC = 3 * HPC * D  # projection output rows per core (768)

PDT_NAME = os.environ.get("SA_PDT", "bf16")

_PROGRAM_CACHE = {}


def _build_program(pdt_name):
    import concourse.mybir as mybir
    from concourse import bacc
    import concourse.tile as tile
    from contextlib import ExitStack

    f32 = mybir.dt.float32
    bf16 = mybir.dt.bfloat16
    Exp = mybir.ActivationFunctionType.Exp

    nc = bacc.Bacc()
    xT_d = nc.declare_dram_parameter("xT", [C, T], bf16, isOutput=False)
    wT_d = nc.declare_dram_parameter("wT", [C, OPC], bf16, isOutput=False)
    msk_d = nc.declare_dram_parameter("msk", [128, 2, 128], bf16, isOutput=False)
    out_d = nc.declare_dram_parameter("out", [T, HPC * D], f32, isOutput=True)

    CC = C // 128  # 8 contraction chunks
    TS = 512  # max projection t-slice (PSUM bank cap)
    NB = T // 128  # 16 query/key blocks
    # variable t-slices: small first (compute starts sooner after less DMA)
    # and small last (fewer attention chunks left for the serial tail)
    SLICES = [(0, 256), (256, 256), (512, 512), (1024, 512), (1536, 256), (1792, 256)]

    with ExitStack() as ctx:
        tc = ctx.enter_context(tile.TileContext(nc))
        const = ctx.enter_context(tc.tile_pool(name="const", bufs=1))
        xpool = ctx.enter_context(tc.tile_pool(name="xp", bufs=3))
        ppool = ctx.enter_context(tc.tile_pool(name="pp", bufs=5))
        opool = ctx.enter_context(tc.tile_pool(name="op", bufs=2))
        rpool = ctx.enter_context(tc.tile_pool(name="rp", bufs=2))
        pj_ps = ctx.enter_context(tc.tile_pool(name="pjps", bufs=2, space="PSUM"))
        sc_ps = ctx.enter_context(tc.tile_pool(name="scps", bufs=2, space="PSUM"))
        ov_ps = ctx.enter_context(tc.tile_pool(name="ovps", bufs=2, space="PSUM"))

        # ---- constants / persistent SBUF ----
        w_sb = const.tile([128, CC, OPC], bf16)
        wT_r = wT_d.rearrange("(cc p) o -> p cc o", p=128)
        msk_sb = const.tile([128, 2, 128], bf16)
        # HAM prewarm: zeros tile + dummy matmuls keep the PE busy during
        # input staging so the clock gate is at 8/8 when real work arrives
        z_sb = const.tile([128, TS], bf16)
        nc.vector.memset(z_sb, 0.0)

        q_sb = const.tile([128, 2, T], bf16)
        k_sb = const.tile([128, 2, T], bf16)
        # v packed per (key block, head) with a trailing ones column
        v_sb = const.tile([128, NB, HPC, D + 1], bf16)
        nc.vector.memset(v_sb[:, :, :, D:D + 1], 1.0)

        xT_r = xT_d.rearrange("(cc p) t -> p cc t", p=128)

        xs_tiles = {}

        def emit_x_dma(s):
            st, ts = SLICES[s]
            xs = xpool.tile([128, CC, TS], bf16, tag="xs")
            nc.sync.dma_start(out=xs[:, :, 0:ts], in_=xT_r[:, :, st:st + ts])
            xs_tiles[s] = xs

        def emit_proj_qk_group(s, m):
            """One output m-tile (128 rows of q|k) for t-slice s."""
            st, ts = SLICES[s]
            xs = xs_tiles[s]
            ps = pj_ps.tile([128, TS], f32, tag="pj")
            for c in range(CC):
                nc.tensor.matmul(
                    ps[:, 0:ts],
                    lhsT=w_sb[:, c, m * 128:(m + 1) * 128],
                    rhs=xs[:, c, 0:ts],
                    start=(c == 0),
                    stop=(c == CC - 1),
                )
            dst = (q_sb if m < 2 else k_sb)[:, m % 2, st:st + ts]
            # balance PSUM evictions: q tiles on ACT, k tiles on DVE
            if m < 2:
                nc.scalar.copy(dst, ps[:, 0:ts])
            else:
                nc.vector.tensor_copy(dst, ps[:, 0:ts])

        def emit_proj_v_group(s, t4):
            """One key block (128 tokens) of v for t-slice s."""
            st, ts = SLICES[s]
            xs = xs_tiles[s]
            ps = pj_ps.tile([128, TS], f32, tag="pj")
            pv = ps[:, 0:HPC * D]
            for c in range(CC):
                nc.tensor.matmul(
                    pv,
                    lhsT=xs[:, c, t4 * 128:(t4 + 1) * 128],
                    rhs=w_sb[:, c, 2 * HPC * D:3 * HPC * D],
                    start=(c == 0),
                    stop=(c == CC - 1),
                )
            tb = st // 128 + t4
            nc.vector.tensor_copy(
                v_sb[:, tb, :, 0:D], pv.rearrange("p (h d) -> p h d", h=HPC)
            )

        p_tiles = {}

        def emit_attn_chunk(jb):
            """scores^T + exp + mask for key chunk jb (all 4 heads)."""
            blo = max(jb - 1, 0)
            bhi = min(jb + 2, NB)
            qlo, qhi = blo * 128, bhi * 128
            n = qhi - qlo
            off = 128 if jb == 0 else 0  # slot offset * 128
            slo = 1 if jb == 0 else 0
            shi = slo + (n // 128)
            p_t = ppool.tile([128, HPC, 3, 128], bf16, tag="p")
            # separate PSUM tile + exp per head pair (mt): halves the
            # score->exp->next-score serialization latency in the tail
            for mt in range(2):
                sct = sc_ps.tile([128, 2, TS], f32, tag="sc")
                for ph in range(2):
                    nc.tensor.matmul(
                        sct[:, ph, off:off + n],
                        lhsT=k_sb[ph * 64:(ph + 1) * 64, mt, jb * 128:(jb + 1) * 128],
                        rhs=q_sb[ph * 64:(ph + 1) * 64, mt, qlo:qhi],
                        start=True,
                        stop=True,
                    )
                nc.scalar.activation(
                    p_t[:, 2 * mt:2 * mt + 2, slo:shi, :],
                    sct[:, :, off:off + n].rearrange("p h (s q) -> p h s q", q=128),
                    Exp,
                )
            # window mask on the edge slots (slot 0: queries of block jb-1,
            # slot 2: queries of block jb+1); middle slot always in-window
            if jb == 0:
                nc.vector.tensor_mul(
                    p_t[:, :, 2, :],
                    p_t[:, :, 2, :],
                    msk_sb[:, 1:2, :].broadcast_to([128, HPC, 128]),
                )
            elif jb == NB - 1:
                nc.vector.tensor_mul(
                    p_t[:, :, 0, :],
                    p_t[:, :, 0, :],
                    msk_sb[:, 0:1, :].broadcast_to([128, HPC, 128]),
                )
            else:
                nc.vector.tensor_mul(
                    p_t[:, :, 0:3:2, :],
                    p_t[:, :, 0:3:2, :],
                    msk_sb[:, :, :].unsqueeze(1).broadcast_to([128, HPC, 2, 128]),
                )
            p_tiles[jb] = p_t

        def emit_pv_block(i):
            """P @ [v|1] for query block i + normalize + store."""
            jbs = [jb for jb in (i - 1, i, i + 1) if 0 <= jb < NB]
            ov = ov_ps.tile([128, HPC, D + 1], f32, tag="ov")
            for h in range(HPC):
                for ci, jb in enumerate(jbs):
                    slot = i - jb + 1
                    nc.tensor.matmul(
                        ov[:, h, :],
                        lhsT=p_tiles[jb][:, h, slot, :],
                        rhs=v_sb[:, jb, h, :],
                        start=(ci == 0),
                        stop=(ci == len(jbs) - 1),
                    )
            r_t = rpool.tile([128, HPC], f32, tag="r")
            nc.vector.reciprocal(r_t, ov[:, :, D])
            o_t = opool.tile([128, HPC, D], f32, tag="o")
            if i == NB - 2:
                # second-to-last block normalizes on ACT so the final two
                # blocks' evictions overlap instead of serializing on DVE
                for h in range(HPC):
                    nc.scalar.mul(o_t[:, h, :], ov[:, h, 0:D], r_t[:, h:h + 1])
            else:
                nc.vector.tensor_mul(
                    o_t,
                    ov[:, :, 0:D],
                    r_t[:, :].unsqueeze(2).broadcast_to([128, HPC, D]),
                )
            eng = nc.gpsimd if i % 2 == 0 else nc.sync
            eng.dma_start(out=out_d[i * 128:(i + 1) * 128, :], in_=o_t)

        # ---- schedule ----
        # attention chunk-steps for key chunk jb are interleaved into the
        # projection of the first slice after jb's inputs are complete;
        # the last 3 chunks run after all projection work.
        attn_of_slice = {
            1: [0], 2: [1, 2], 3: [3, 4, 5, 6], 4: [7, 8, 9, 10], 5: [11, 12],
        }
        tail_chunks = [13, 14, 15]
        PV_LAG = 3

        next_pv = [0]

        def emit_attn_step(jb):
            emit_attn_chunk(jb)
            i = jb - PV_LAG
            while next_pv[0] <= i:
                emit_pv_block(next_pv[0])
                next_pv[0] += 1

        # input staging: only w m-tile 0 + xs0 gate the first matmul; defer
        # the rest of w so xs0 doesn't compete for DMA bandwidth
        nc.sync.dma_start(out=w_sb[:, :, 0:128], in_=wT_r[:, :, 0:128])
        emit_x_dma(0)
        nc.sync.dma_start(out=w_sb[:, :, 128:384], in_=wT_r[:, :, 128:384])
        emit_x_dma(1)
        nc.sync.dma_start(out=w_sb[:, :, 384:768], in_=wT_r[:, :, 384:768])
        nc.gpsimd.dma_start(out=msk_sb, in_=msk_d[:, :, :])
        # HAM prewarm: dummy matmuls bridge the staging window (and its DMA
        # hiccups) with PE activity so the clock gate reaches and HOLDS 8/8
        # before the real matmul stream begins
        warm = sc_ps.tile([128, 2, TS], f32, tag="sc")
        for _ in range(12):
            nc.tensor.matmul(
                warm[:, 0, :], lhsT=z_sb[:, 0:128], rhs=z_sb, start=True, stop=True
            )

        for s in range(len(SLICES)):
            if s + 2 < len(SLICES):
                emit_x_dma(s + 2)
            steps = list(attn_of_slice.get(s, []))
            nsteps = len(steps)
            nt4 = SLICES[s][1] // 128
            groups = [("qk", m) for m in range(4)] + [("v", t4) for t4 in range(nt4)]
            ng = len(groups)
            for gi, (kind, idx) in enumerate(groups):
                if kind == "qk":
                    emit_proj_qk_group(s, idx)
                else:
                    emit_proj_v_group(s, idx)
                # spread this slice's attention steps evenly between groups
                target = (nsteps * (gi + 1) + ng - 1) // ng
                while len(steps) > nsteps - target:
                    emit_attn_step(steps.pop(0))
            while steps:
                emit_attn_step(steps.pop(0))
        for jb in tail_chunks:
            emit_attn_step(jb)
        while next_pv[0] < NB:
            emit_pv_block(next_pv[0])
            next_pv[0] += 1

    nc.compile()
    return nc


def _host_inputs(x, Wqkv):
    """Per-core input maps: shard batch x head-group, pre-transpose, bf16."""
    import concourse.mybir as mybir

    bf16_np = mybir.dt.np(mybir.dt.bfloat16)
    scale = float(D) ** -0.5
    r = np.arange(128, dtype=np.float32)[:, None]
    ci = np.arange(128, dtype=np.float32)[None, :]
    # slot 0 (queries of block jb-1): allowed iff key row r <= query col c
    # slot 2 (queries of block jb+1): allowed iff c <= r
    msk = np.stack(
        [
            (ci >= r).astype(np.float32),
            (ci <= r).astype(np.float32),
        ],
        axis=1,
    ).astype(bf16_np)  # [128, 2, 128]

    x = np.asarray(x, dtype=np.float32)
    Wqkv = np.asarray(Wqkv, dtype=np.float32)
    xT = [np.ascontiguousarray(x[b].T).astype(bf16_np) for b in range(B)]
    in_maps = []
    for core in range(N_CORES):
        b, hg = divmod(core, N_CORES // B)
        rows = slice(hg * HPC * D, (hg + 1) * HPC * D)
        wcat = np.concatenate(
            [
                Wqkv[0 * C:1 * C][rows] * scale,
                Wqkv[1 * C:2 * C][rows],
                Wqkv[2 * C:3 * C][rows],
            ],
            axis=0,
        )
        in_maps.append(
            {
                "xT": xT[b],
                "wT": np.ascontiguousarray(wcat.T).astype(bf16_np),
                "msk": msk,
            }
        )
    return in_maps


def _gather(results):
    out = np.empty((B, T, C), dtype=np.float32)
    for core in range(N_CORES):
        b, hg = divmod(core, N_CORES // B)
        out[b, :, hg * HPC * D:(hg + 1) * HPC * D] = results[core]["out"]
    return out


def kernel(x, Wqkv):
    from concourse.bass_utils import run_bass_kernel_spmd

    key = PDT_NAME
    if key not in _PROGRAM_CACHE:
        _PROGRAM_CACHE[key] = _build_program(key)
    nc = _PROGRAM_CACHE[key]
    in_maps = _host_inputs(x, Wqkv)
    res = run_bass_kernel_spmd(nc, in_maps, list(range(N_CORES)))
    return _gather(res.results)
